# revision 11
# baseline (speedup 1.0000x reference)
"""Windowed (block-local) multi-head attention on 8 Trainium2 NeuronCores.

Reference computation (fp32):
    x:[B=2, T=8192, C=1024], w_qkv:[3C, C], w_out:[C, C]
    per window of W=512 rows: qkv projection, per-head (H=16, D=64)
    softmax(q k^T / 8) v, then output projection.

Sharding: the 32 (B*nW) independent windows are split 4-per-core
(sequence parallel, zero communication). Weights are replicated.

Host-side prep (free — not on the device clock): shard, transpose to
"contraction dim on partitions" layouts, cast to bf16.

Device dataflow per window (all matmuls bf16, fp32 PSUM accumulate):
  qkT  [f, w] = w_qk @ x^T      (16 f-chunks x 8 k-chunks, N=512)
  v    [w, f]  = x @ w_v^T       (4 w-chunks x 2 o-tiles x 8 k-chunks)
  per head h:
    scoresT[j, i] = k_h^T q_h    (4 j-chunks, K=64, N=512) -> one PSUM [128,4,512]
    expT = exp(0.125 * scoresT)  (single ACT op, PSUM->SBUF bf16)
    out2T_aug[0:64,i], s[i] = [v_h | 1]^T @ expT  (augmented-V: softmax
        denominator falls out of row 64 of the same accumulating matmul)
    s broadcast to 64 partitions via stride-0 DMA; DVE divide -> out2T bf16
  y    [w, o] = out2^T^T @ w_out^T  (accumulate 8 c-chunks) -> DMA to DRAM fp32
"""

import os

import numpy as np
import ml_dtypes

import concourse.bass as bass
import concourse.tile as tile
import concourse.mybir as mybir
from concourse import bacc
from concourse.bass_utils import run_bass_kernel_spmd

BF16 = mybir.dt.bfloat16
F32 = mybir.dt.float32

B, T, C = 2, 8192, 1024
H, D, W = 16, 64, 512
NW = T // W          # 16 windows per batch element
NCORES = 8
NWPC = B * NW // NCORES  # 4 windows per core
CC = C // 128        # 8 contraction chunks
P = 128

_cache = {}


def _build_kernel(reps=1):
    nc = bacc.Bacc("TRN2", target_bir_lowering=False, debug=False)

    xT_d = nc.dram_tensor("xT", [NWPC, P, CC, W], BF16, kind="ExternalInput").ap()
    wqkT_d = nc.dram_tensor("wqkT", [P, CC, 2 * C], BF16, kind="ExternalInput").ap()
    wvT_d = nc.dram_tensor("wvT", [P, CC, C], BF16, kind="ExternalInput").ap()
    woutT_d = nc.dram_tensor("woutT", [P, CC, C], BF16, kind="ExternalInput").ap()
    y_d = nc.dram_tensor("y", [NWPC, W, C], F32, kind="ExternalOutput").ap()

    with tile.TileContext(nc) as tc:
        with (
            tc.tile_pool(name="wconst", bufs=1) as wpool,
            tc.tile_pool(name="xt", bufs=2) as xpool,
            tc.tile_pool(name="qk", bufs=2) as qkpool,
            tc.tile_pool(name="vp", bufs=2) as vpool,
            tc.tile_pool(name="expp", bufs=3) as epool,
            tc.tile_pool(name="yin", bufs=2) as ypool,
            tc.tile_pool(name="sbc", bufs=4) as spool,
            tc.tile_pool(name="ps_mm", bufs=2, space="PSUM") as ps_mm,
            tc.tile_pool(name="ps_sc", bufs=1, space="PSUM") as ps_sc,
            tc.tile_pool(name="ps_av", bufs=2, space="PSUM") as ps_av,
        ):
            wqk_sb = wpool.tile([P, CC, 2 * C], BF16)
            nc.sync.dma_start(wqk_sb[:], wqkT_d[:])
            wv_sb = wpool.tile([P, CC, C], BF16)
            nc.sync.dma_start(wv_sb[:], wvT_d[:])
            wout_sb = wpool.tile([P, CC, C], BF16)
            nc.sync.dma_start(wout_sb[:], woutT_d[:])

            if reps > 1:
                rep_ctx = tc.For_i(0, reps, 1)
                rep_ctx.__enter__()

            for win in range(NWPC):
                xt = xpool.tile([P, CC, W], BF16)
                nc.sync.dma_start(xt[:], xT_d[win])

                # ---- QK projection: qkT[f, w] for f in [0, 2C) ----
                qkT = qkpool.tile([P, 16, W], BF16)
                for fc in range(16):
                    ps = ps_mm.tile([P, W], F32, tag="mm")
                    for cc in range(CC):
                        nc.tensor.matmul(
                            ps[:],
                            wqk_sb[:, cc, fc * P:(fc + 1) * P],
                            xt[:, cc, :],
                            start=(cc == 0),
                            stop=(cc == CC - 1),
                        )
                    nc.scalar.copy(qkT[:, fc, :], ps[:])

                # ---- V projection: v[w, f] with ones column appended ----
                v_sb = vpool.tile([P, 4, H, D + 1], BF16)
                nc.vector.memset(v_sb[:, :, :, D:D + 1], 1.0)
                for wc in range(4):
                    for ot in range(2):
                        ps = ps_mm.tile([P, W], F32, tag="mm")
                        for cc in range(CC):
                            nc.tensor.matmul(
                                ps[:],
                                xt[:, cc, wc * P:(wc + 1) * P],
                                wv_sb[:, cc, ot * 512:(ot + 1) * 512],
                                start=(cc == 0),
                                stop=(cc == CC - 1),
                            )
                        nc.vector.tensor_copy(
                            v_sb[:, wc, ot * 8:(ot + 1) * 8, 0:D],
                            ps.rearrange("p (h d) -> p h d", d=D),
                        )

                # ---- attention per head ----
                y_in = ypool.tile([P, CC, W], BF16)
                for h in range(H):
                    qT = qkT[(h % 2) * D:(h % 2) * D + D, h // 2, :]
                    kT = qkT[(h % 2) * D:(h % 2) * D + D, 8 + h // 2, :]

                    ps_s = ps_sc.tile([P, 4, W], F32, tag="sc")
                    for jc in range(4):
                        nc.tensor.matmul(
                            ps_s[:, jc, :],
                            kT[:, jc * P:(jc + 1) * P],
                            qT,
                            start=True,
                            stop=True,
                        )

                    expT = epool.tile([P, 4, W], BF16, tag="expT")
                    nc.scalar.activation(
                        expT[:], ps_s[:],
                        mybir.ActivationFunctionType.Exp,
                        scale=0.125,
                    )

                    ps_o = ps_av.tile([D + 1, W], F32, tag="av")
                    for jc in range(4):
                        nc.tensor.matmul(
                            ps_o[:],
                            v_sb[:, jc, h, :],
                            expT[:, jc, :],
                            start=(jc == 0),
                            stop=(jc == 3),
                        )

                    # broadcast the softmax denominator (row D) to 64
                    # partitions, then a single divide normalizes out2T
                    s_row = spool.tile([1, W], F32, tag="srow")
                    nc.vector.reciprocal(s_row[:], ps_o[D:D + 1, :])
                    s_bc = spool.tile([D, W], F32, tag="sbc")
                    nc.gpsimd.partition_broadcast(s_bc[:], s_row[:])
                    nc.vector.tensor_mul(
                        y_in[(h % 2) * D:(h % 2) * D + D, h // 2, :],
                        ps_o[0:D, :],
                        s_bc[:],
                    )

                # ---- output projection ----
                for wc in range(4):
                    for ot in range(2):
                        ps = ps_mm.tile([P, W], F32, tag="mm")
                        for cc in range(CC):
                            nc.tensor.matmul(
                                ps[:],
                                y_in[:, cc, wc * P:(wc + 1) * P],
                                wout_sb[:, cc, ot * 512:(ot + 1) * 512],
                                start=(cc == 0),
                                stop=(cc == CC - 1),
                            )
                        y_sb = spool.tile([P, W], F32, tag="ysb")
                        nc.scalar.copy(y_sb[:], ps[:])
                        nc.sync.dma_start(
                            y_d[win, wc * P:(wc + 1) * P, ot * 512:(ot + 1) * 512],
                            y_sb[:],
                        )

            if reps > 1:
                rep_ctx.__exit__(None, None, None)
    nc.compile()
    return nc


def _prep_inputs(x, w_qkv, w_out):
    bf16 = ml_dtypes.bfloat16
    # x -> per-window transposed [32, 128, 8, 512] (c on partitions)
    xw = np.asarray(x, np.float32).reshape(B * NW, W, C)
    xT = xw.transpose(0, 2, 1).reshape(B * NW, CC, P, W)
    xT = np.ascontiguousarray(xT.transpose(0, 2, 1, 3)).astype(bf16)

    def prep_w(wt):  # [F, C] -> [128, CC, F] with c = cc*128 + ci
        t = np.asarray(wt, np.float32).T  # [C, F]
        t = t.reshape(CC, P, -1).transpose(1, 0, 2)
        return np.ascontiguousarray(t).astype(bf16)

    wqkT = prep_w(w_qkv[:2 * C])
    wvT = prep_w(w_qkv[2 * C:])
    woutT = prep_w(w_out)
    return xT, wqkT, wvT, woutT


def kernel(x, w_qkv, w_out):
    if "nc" not in _cache:
        _cache["nc"] = _build_kernel()
    nc = _cache["nc"]

    xT, wqkT, wvT, woutT = _prep_inputs(x, w_qkv, w_out)
    in_maps = [
        {
            "xT": np.ascontiguousarray(xT[c * NWPC:(c + 1) * NWPC]),
            "wqkT": wqkT,
            "wvT": wvT,
            "woutT": woutT,
        }
        for c in range(NCORES)
    ]
    trace = os.environ.get("KERNEL_TRACE", "0") == "1"
    res = run_bass_kernel_spmd(nc, in_maps, list(range(NCORES)), trace=trace)
    if trace:
        _cache["last_results"] = res

    y = np.concatenate([res.results[c]["y"] for c in range(NCORES)], axis=0)
    return y.reshape(B, T, C).astype(np.float32)
